# revision 4
# baseline (speedup 1.0000x reference)
"""Trainium2 Bass kernel for a 3-layer KAN (Kolmogorov-Arnold Network).

Math: each layer is  y = clip(silu(x) @ bw.T + einsum('bik,oik->bo', B3bases(x), sw), -1, 1)
with 11 cubic B-spline bases on centers linspace(-1.25, 1.25, 11), grid 0.25.

Reformulation (exact): with s = 4x+5, the cardinal cubic B-spline obeys
  B3(s-k) = (1/6) * sum_m [1,-4,6,-4,1][m] * relu(s-k+2-m)^3
so every basis value is a fixed linear combination of shifted relu-cubes
t_j = relu(s-j)^3, j=-2..8 (t_j = 0 for j>=9 since x in [-1,1]).  For j<=1,
t_j is a pure cubic (relu inactive); for j=2,3,4 we mirror
(relu(v)^3 = v^3 + relu(-v)^3) to keep channel values bounded.  Folding the
linear combination into the spline weights, a layer becomes ONE dense matmul
over 11 channels per input feature:
  phi = [silu(x), x, x^2, x^3, relu(-4x-3)^3, relu(-4x-2)^3, relu(-4x-1)^3,
         relu(4x)^3, relu(4x-1)^3, relu(4x-2)^3, relu(4x-3)^3]
  y = clip(phi @ Wfold + bias, -1, 1)
Channel values are bounded (<=64), so bf16 matmul with fp32 PSUM accumulation
gives ~3e-4 end-to-end relative error.

Distribution: data-parallel over 8 cores (batch 8192 -> 1024/core), weights
replicated and streamed from HBM.  Activations kept feature-major [fin, B]
so the matmul output [fout, n] directly feeds the next layer with no
transposes on device.  Batch processed in 2 chunks of 512 (PSUM capacity).
"""

import numpy as np
import ml_dtypes

import concourse.bacc as bacc
import concourse.mybir as mybir
import concourse.tile as tile
from concourse.bass_utils import run_bass_kernel_spmd

# ---------------- problem constants (hardcoded) ----------------
B_FULL = 8192
LAYERS = [512, 1024, 1024, 256]
N_CORES = 8
BS = B_FULL // N_CORES          # 1024 batch rows per core
N_CHUNK = 2                     # batch chunks per core
NB = BS // N_CHUNK              # 512 batch per chunk
NCH = 11                        # channels per input feature
F_BLK = 2                       # fin-tiles per wide activation tile

FP32 = mybir.dt.float32
BF16 = mybir.dt.bfloat16
AF = mybir.ActivationFunctionType
ALU = mybir.AluOpType

# relu-cube channels 4..10: r = relu(a*x + b)
J_PARAMS = [(-4.0, -3.0), (-4.0, -2.0), (-4.0, -1.0),
            (4.0, 0.0), (4.0, -1.0), (4.0, -2.0), (4.0, -3.0)]


# ---------------- host-side weight folding ----------------
def _fold_weights(bw, sw):
    """bw [fout, fin] f32, sw [fout, fin, 11] f32 ->
    (wtiles [n_k, 128, fout] bf16, bias_t [128, n_m] f32)"""
    bw = bw.astype(np.float64)
    sw = sw.astype(np.float64)
    fout, fin, K = sw.shape
    c = np.array([1.0, -4.0, 6.0, -4.0, 1.0], dtype=np.float64) / 6.0
    # G[o,i,j] coefficients on t_j = relu(s-j)^3, j=-2..8 (idx j+2)
    G = np.zeros((fout, fin, 11), dtype=np.float64)
    for k in range(11):
        for m in range(5):
            j = k - 2 + m
            if -2 <= j <= 8:
                G[:, :, j + 2] += sw[:, :, k] * c[m]
    # polynomial part from j=-2..4: (4x + (5-j))^3
    Wd = np.zeros((4, fout, fin), dtype=np.float64)
    for j in range(-2, 5):
        b = 5.0 - j
        beta = (b ** 3, 12.0 * b * b, 48.0 * b, 64.0)
        for d in range(4):
            Wd[d] += G[:, :, j + 2] * beta[d]
    Wc = np.empty((NCH, fout, fin), dtype=np.float64)
    Wc[0] = bw
    Wc[1] = Wd[1]
    Wc[2] = Wd[2]
    Wc[3] = Wd[3]
    for t in range(7):          # channels 4..10 <- G j=2..8 (idx 4..10)
        Wc[4 + t] = G[:, :, 4 + t]
    bias = Wd[0].sum(axis=1)    # [fout]

    F = fin // 128
    n_m = fout // 128
    # wtiles[f*11 + ch, p, o] = Wc[ch, o, f*128+p]
    wt = Wc.reshape(NCH, fout, F, 128).transpose(2, 0, 3, 1)  # [F, NCH, 128, fout]
    wt = np.ascontiguousarray(wt.reshape(F * NCH, 128, fout)).astype(ml_dtypes.bfloat16)
    bias_t = np.ascontiguousarray(bias.reshape(n_m, 128).T).astype(np.float32)  # [128, n_m]
    return wt, bias_t


# ---------------- device program ----------------
_NC_CACHE = {}


def _build_program():
    if "nc" in _NC_CACHE:
        return _NC_CACHE["nc"]

    nc = bacc.Bacc("TRN2", target_bir_lowering=False, debug=False,
                   num_devices=N_CORES)

    xt_dram = nc.dram_tensor("xt", [LAYERS[0], BS], FP32, kind="ExternalInput")
    w_dram, b_dram = [], []
    for l in range(3):
        fin, fout = LAYERS[l], LAYERS[l + 1]
        n_k = (fin // 128) * NCH
        n_m = fout // 128
        w_dram.append(nc.dram_tensor(f"w{l}", [n_k, 128, n_m * 128], BF16,
                                     kind="ExternalInput"))
        b_dram.append(nc.dram_tensor(f"b{l}", [128, n_m], FP32,
                                     kind="ExternalInput"))
    out_dram = nc.dram_tensor("out", [LAYERS[3], BS], FP32, kind="ExternalOutput")

    with tile.TileContext(nc) as tc:
        with (
            tc.tile_pool(name="xp", bufs=8) as xp,
            tc.tile_pool(name="chp", bufs=24) as chp,
            tc.tile_pool(name="wp", bufs=8) as wp,
            tc.tile_pool(name="tmpp", bufs=6) as tmpp,
            tc.tile_pool(name="ostp", bufs=4) as ostp,
            tc.tile_pool(name="biasp", bufs=3) as biasp,
            tc.tile_pool(name="psump", bufs=8, space="PSUM") as psump,
        ):
            # const bias tiles for ACT relu shifts (-1, -2, -3)
            cmap = {0.0: 0.0}
            for v in (-1.0, -2.0, -3.0):
                ct = biasp.tile([128, 1], FP32, name=f"cn{int(-v)}", tag="const")
                nc.vector.memset(ct[:], v)
                cmap[v] = ct

            bias_sb = []
            for l in range(3):
                n_m = LAYERS[l + 1] // 128
                bt = biasp.tile([128, n_m], FP32, tag="bias")
                nc.sync.dma_start(bt[:], b_dram[l][:])
                bias_sb.append(bt)

            for cix in range(N_CHUNK):
                nsl = slice(cix * NB, (cix + 1) * NB)
                # ---- load layer-0 input tiles (wide: F_BLK fin-tiles) ----
                xw = []
                for fb in range(LAYERS[0] // 128 // F_BLK):
                    t = xp.tile([128, F_BLK * NB], FP32, tag="x")
                    for fl in range(F_BLK):
                        f = fb * F_BLK + fl
                        nc.sync.dma_start(
                            t[:, fl * NB:(fl + 1) * NB],
                            xt_dram[f * 128:(f + 1) * 128, nsl])
                    xw.append(t)

                for l in range(3):
                    fin, fout = LAYERS[l], LAYERS[l + 1]
                    F = fin // 128
                    n_k = F * NCH
                    n_m = fout // 128

                    psums = [psump.tile([128, NB], FP32, tag="ps", name=f"ps{m}")
                             for m in range(n_m)]

                    for fb in range(F // F_BLK):
                        xf = xw[fb]
                        ch = [None] * NCH
                        # channel 1: x (bf16 cast)
                        c1 = chp.tile([128, F_BLK * NB], BF16, tag="ch")
                        nc.vector.tensor_copy(c1[:], xf[:])
                        ch[1] = c1
                        # channel 0: silu(x)
                        c0 = chp.tile([128, F_BLK * NB], BF16, tag="ch")
                        nc.scalar.activation(c0[:], xf[:], AF.Silu)
                        ch[0] = c0
                        # channels 2, 3: x^2, x^3
                        c2 = chp.tile([128, F_BLK * NB], BF16, tag="ch")
                        nc.vector.tensor_tensor(c2[:], c1[:], c1[:], ALU.mult)
                        ch[2] = c2
                        c3 = chp.tile([128, F_BLK * NB], BF16, tag="ch")
                        nc.vector.tensor_tensor(c3[:], c2[:], c1[:], ALU.mult)
                        ch[3] = c3
                        # channels 4..10: relu(a*x+b)^3
                        for t7, (a, b) in enumerate(J_PARAMS):
                            r = chp.tile([128, F_BLK * NB], BF16, tag="scr")
                            bias_arg = 0.0 if b == 0.0 else cmap[b][:]
                            nc.scalar.activation(r[:], xf[:], AF.Relu,
                                                 bias=bias_arg, scale=a)
                            q = chp.tile([128, F_BLK * NB], BF16, tag="scr")
                            nc.vector.tensor_tensor(q[:], r[:], r[:], ALU.mult)
                            rho = chp.tile([128, F_BLK * NB], BF16, tag="ch")
                            nc.vector.tensor_tensor(rho[:], q[:], r[:], ALU.mult)
                            ch[4 + t7] = rho

                        # ---- matmuls for the F_BLK fin-tiles of this block ----
                        for fl in range(F_BLK):
                            f = fb * F_BLK + fl
                            for cidx in range(NCH):
                                k = f * NCH + cidx
                                wt = wp.tile([128, n_m * 128], BF16, tag="w")
                                nc.sync.dma_start(wt[:], w_dram[l][k])
                                rhs = ch[cidx][:, fl * NB:(fl + 1) * NB]
                                for m in range(n_m):
                                    nc.tensor.matmul(
                                        psums[m][:],
                                        wt[:, m * 128:(m + 1) * 128],
                                        rhs,
                                        start=(k == 0),
                                        stop=(k == n_k - 1))

                    # ---- drain: bias add + clip ----
                    if l < 2:
                        xw_next = [xp.tile([128, F_BLK * NB], FP32, tag="x", name=f"xn{i}")
                                   for i in range(n_m // F_BLK)]
                    for m in range(n_m):
                        tmp = tmpp.tile([128, NB], FP32, tag="tmp")
                        nc.scalar.activation(tmp[:], psums[m][:], AF.Identity,
                                             bias=bias_sb[l][:, m:m + 1])
                        if l < 2:
                            dst = xw_next[m // F_BLK][:, (m % F_BLK) * NB:
                                                      (m % F_BLK + 1) * NB]
                            nc.vector.tensor_scalar(dst, tmp[:], 1.0, -1.0,
                                                    ALU.min, ALU.max)
                        else:
                            o = ostp.tile([128, NB], FP32, tag="ost")
                            nc.vector.tensor_scalar(o[:], tmp[:], 1.0, -1.0,
                                                    ALU.min, ALU.max)
                            nc.sync.dma_start(
                                out_dram[m * 128:(m + 1) * 128, nsl], o[:])
                    if l < 2:
                        xw = xw_next

    nc.compile()
    _NC_CACHE["nc"] = nc
    return nc


# ---------------- entry point ----------------
def kernel(x, base_w0, spline_w0, base_w1, spline_w1, base_w2, spline_w2):
    x = np.asarray(x, dtype=np.float32)
    folded = [
        _fold_weights(np.asarray(base_w0), np.asarray(spline_w0)),
        _fold_weights(np.asarray(base_w1), np.asarray(spline_w1)),
        _fold_weights(np.asarray(base_w2), np.asarray(spline_w2)),
    ]

    nc = _build_program()

    in_maps = []
    for core in range(N_CORES):
        shard = x[core * BS:(core + 1) * BS]                  # [BS, 512]
        xt = np.ascontiguousarray(shard.T)                    # [512, BS]
        m = {"xt": xt}
        for l in range(3):
            m[f"w{l}"] = folded[l][0]
            m[f"b{l}"] = folded[l][1]
        in_maps.append(m)

    res = run_bass_kernel_spmd(nc, in_maps, list(range(N_CORES)))
    out = np.concatenate(
        [np.ascontiguousarray(res.results[i]["out"].T) for i in range(N_CORES)],
        axis=0)                                               # [8192, 256]
    return out.astype(np.float32)


# revision 5
# speedup vs baseline: 139.5256x; 139.5256x over previous
"""Trainium2 Bass kernel for a 3-layer KAN (Kolmogorov-Arnold Network).

Math: each layer is  y = clip(silu(x) @ bw.T + einsum('bik,oik->bo', B3bases(x), sw), -1, 1)
with 11 cubic B-spline bases on centers linspace(-1.25, 1.25, 11), grid 0.25.

Reformulation (exact): with s = 4x+5, the cardinal cubic B-spline obeys
  B3(s-k) = (1/6) * sum_m [1,-4,6,-4,1][m] * relu(s-k+2-m)^3
so every basis value is a fixed linear combination of shifted relu-cubes
t_j = relu(s-j)^3, j=-2..8 (t_j = 0 for j>=9 since x in [-1,1]).  For j<=1,
t_j is a pure cubic (relu inactive); for j=2,3,4 we mirror
(relu(v)^3 = v^3 + relu(-v)^3) to keep channel values bounded.  Folding the
linear combination into the spline weights, a layer becomes ONE dense matmul
over 11 channels per input feature:
  phi = [silu(x), x, x^2, x^3, relu(-4x-3)^3, relu(-4x-2)^3, relu(-4x-1)^3,
         relu(4x)^3, relu(4x-1)^3, relu(4x-2)^3, relu(4x-3)^3]
  y = clip(phi @ Wfold + bias, -1, 1)
Channel values are bounded (<=64), so low-precision matmul with fp32 PSUM
accumulation is accurate: bf16 ~3e-4, fp8e4m3 ~7e-3 end-to-end rel error.

fp8 path: weights scaled by 128 (avoids e4m3 subnormal underflow; inverted in
the drain's activation scale), matmuls use DoubleRow perf mode pairing the two
fin-tiles of each wide activation tile -> one MM contracts 256 rows.

Distribution: data-parallel over 8 cores (batch 8192 -> 1024/core), weights
replicated and streamed from HBM.  Activations kept feature-major [fin, B]
so the matmul output [fout, n] directly feeds the next layer with no
transposes on device.  Batch processed in 2 chunks of 512 (PSUM capacity).
"""

import numpy as np
import ml_dtypes

import concourse.bacc as bacc
import concourse.mybir as mybir
import concourse.tile as tile
from concourse.bass_utils import run_bass_kernel_spmd

# ---------------- problem constants (hardcoded) ----------------
B_FULL = 8192
LAYERS = [512, 1024, 1024, 256]
N_CORES = 8
BS = B_FULL // N_CORES          # 1024 batch rows per core
N_CHUNK = 2                     # batch chunks per core
NB = BS // N_CHUNK              # 512 batch per chunk
NCH = 11                        # channels per input feature
F_BLK = 2                       # fin-tiles per wide activation tile

PREC = "fp8"                    # "fp8" (DoubleRow) or "bf16"
W_SCALE = 128.0                 # fp8 weight prescale, undone in drain

FP32 = mybir.dt.float32
BF16 = mybir.dt.bfloat16
F8 = mybir.dt.float8e4
AF = mybir.ActivationFunctionType
ALU = mybir.AluOpType

# relu-cube channels 4..10: r = relu(a*x + b)
J_PARAMS = [(-4.0, -3.0), (-4.0, -2.0), (-4.0, -1.0),
            (4.0, 0.0), (4.0, -1.0), (4.0, -2.0), (4.0, -3.0)]


# ---------------- host-side weight folding ----------------
def _fold_weights(bw, sw):
    """bw [fout, fin] f32, sw [fout, fin, 11] f32 ->
    (wtiles, bias_t [128, n_m] f32).
    bf16: wtiles [F*11, 128, fout] bf16
    fp8:  wtiles [(F//2)*11, 128, 2, fout] f8e4m3 scaled by W_SCALE,
          pairing fin-tiles (2fb, 2fb+1) for DoubleRow."""
    bw = bw.astype(np.float64)
    sw = sw.astype(np.float64)
    fout, fin, K = sw.shape
    c = np.array([1.0, -4.0, 6.0, -4.0, 1.0], dtype=np.float64) / 6.0
    # G[o,i,j] coefficients on t_j = relu(s-j)^3, j=-2..8 (idx j+2)
    G = np.zeros((fout, fin, 11), dtype=np.float64)
    for k in range(11):
        for m in range(5):
            j = k - 2 + m
            if -2 <= j <= 8:
                G[:, :, j + 2] += sw[:, :, k] * c[m]
    # polynomial part from j=-2..4: (4x + (5-j))^3
    Wd = np.zeros((4, fout, fin), dtype=np.float64)
    for j in range(-2, 5):
        b = 5.0 - j
        beta = (b ** 3, 12.0 * b * b, 48.0 * b, 64.0)
        for d in range(4):
            Wd[d] += G[:, :, j + 2] * beta[d]
    Wc = np.empty((NCH, fout, fin), dtype=np.float64)
    Wc[0] = bw
    Wc[1] = Wd[1]
    Wc[2] = Wd[2]
    Wc[3] = Wd[3]
    for t in range(7):          # channels 4..10 <- G j=2..8 (idx 4..10)
        Wc[4 + t] = G[:, :, 4 + t]
    bias = Wd[0].sum(axis=1)    # [fout]

    F = fin // 128
    n_m = fout // 128
    if PREC == "bf16":
        # wtiles[f*11 + ch, p, o] = Wc[ch, o, f*128+p]
        wt = Wc.reshape(NCH, fout, F, 128).transpose(2, 0, 3, 1)
        wt = np.ascontiguousarray(wt.reshape(F * NCH, 128, fout))
        wt = wt.astype(ml_dtypes.bfloat16)
    else:
        # paired: wtiles[fb*11 + ch, p, two, o] = Wc[ch, o, (2fb+two)*128+p]
        wtp = (Wc * W_SCALE).reshape(NCH, fout, F // 2, 2, 128)
        wtp = wtp.transpose(2, 0, 4, 3, 1)       # [F//2, NCH, 128, 2, fout]
        wt = np.ascontiguousarray(wtp.reshape((F // 2) * NCH, 128, 2, fout))
        wt = wt.astype(ml_dtypes.float8_e4m3)
    bias_t = np.ascontiguousarray(bias.reshape(n_m, 128).T).astype(np.float32)
    return wt, bias_t


# ---------------- device program ----------------
_NC_CACHE = {}


def _emit_body(nc, pools, tensors):
    xp, chp, wp, tmpp, ostp = (pools[k] for k in ("xp", "chp", "wp", "tmpp", "ostp"))
    psump = pools["psump"]
    xt_dram, w_dram, out_dram = tensors["xt"], tensors["w"], tensors["out"]
    bias_sb, cmap = tensors["bias_sb"], tensors["cmap"]
    fp8 = PREC == "fp8"
    drain_scale = (1.0 / W_SCALE) if fp8 else 1.0

    for cix in range(N_CHUNK):
        nsl = slice(cix * NB, (cix + 1) * NB)
        # ---- load layer-0 input tiles (wide: F_BLK fin-tiles) ----
        xw = []
        for fb in range(LAYERS[0] // 128 // F_BLK):
            t = xp.tile([128, F_BLK * NB], FP32, tag="x", name=f"x0_{cix}_{fb}")
            for fl in range(F_BLK):
                f = fb * F_BLK + fl
                nc.sync.dma_start(
                    t[:, fl * NB:(fl + 1) * NB],
                    xt_dram[f * 128:(f + 1) * 128, nsl])
            xw.append(t)

        for l in range(3):
            fin, fout = LAYERS[l], LAYERS[l + 1]
            F = fin // 128
            n_m = fout // 128
            n_k = F * NCH
            n_kp = (F // 2) * NCH

            psums = [psump.tile([128, NB], FP32, tag="ps", name=f"ps{m}")
                     for m in range(n_m)]

            for fb in range(F // F_BLK):
                xf = xw[fb]
                ch = [None] * NCH
                W = F_BLK * NB
                cdt = F8 if fp8 else BF16
                # x in bf16 (working precision for products)
                xb = chp.tile([128, W], BF16, tag="scr")
                nc.vector.tensor_copy(xb[:], xf[:])
                # channel 0: silu(x)
                c0 = chp.tile([128, W], cdt, tag="ch")
                nc.scalar.activation(c0[:], xf[:], AF.Silu)
                ch[0] = c0
                # channel 1: x
                if fp8:
                    c1 = chp.tile([128, W], cdt, tag="ch")
                    nc.gpsimd.tensor_copy(c1[:], xb[:])
                    ch[1] = c1
                else:
                    ch[1] = xb
                # channels 2, 3: x^2, x^3
                x2b = chp.tile([128, W], BF16, tag="scr")
                nc.vector.tensor_tensor(x2b[:], xb[:], xb[:], ALU.mult)
                if fp8:
                    c2 = chp.tile([128, W], cdt, tag="ch")
                    nc.gpsimd.tensor_copy(c2[:], x2b[:])
                    ch[2] = c2
                else:
                    ch[2] = x2b
                c3 = chp.tile([128, W], cdt, tag="ch")
                nc.vector.tensor_tensor(c3[:], x2b[:], xb[:], ALU.mult)
                ch[3] = c3
                # channels 4..10: relu(a*x+b)^3
                for t7, (a, b) in enumerate(J_PARAMS):
                    r = chp.tile([128, W], BF16, tag="scr")
                    bias_arg = 0.0 if b == 0.0 else cmap[b][:]
                    nc.scalar.activation(r[:], xf[:], AF.Relu,
                                         bias=bias_arg, scale=a)
                    q = chp.tile([128, W], BF16, tag="scr")
                    nc.vector.tensor_tensor(q[:], r[:], r[:], ALU.mult)
                    rho = chp.tile([128, W], cdt, tag="ch")
                    nc.vector.tensor_tensor(rho[:], q[:], r[:], ALU.mult)
                    ch[4 + t7] = rho

                # ---- matmuls ----
                if fp8:
                    # DoubleRow: pair the two fin-tiles of each wide tile
                    for pl in range(F_BLK // 2):
                        for cidx in range(NCH):
                            kp = (fb * (F_BLK // 2) + pl) * NCH + cidx
                            wt = wp.tile([128, 2, n_m * 128], F8, tag="w")
                            nc.sync.dma_start(wt[:], w_dram[l][kp])
                            rhs = ch[cidx][:, pl * 2 * NB:(pl + 1) * 2 * NB]
                            rhs3 = rhs.rearrange("p (two n) -> p two n", two=2)
                            for m in range(n_m):
                                nc.tensor.matmul(
                                    psums[m][:],
                                    wt[:, :, m * 128:(m + 1) * 128],
                                    rhs3,
                                    start=(kp == 0), stop=(kp == n_kp - 1),
                                    perf_mode=mybir.MatmulPerfMode.DoubleRow)
                else:
                    for fl in range(F_BLK):
                        f = fb * F_BLK + fl
                        for cidx in range(NCH):
                            k = f * NCH + cidx
                            wt = wp.tile([128, n_m * 128], BF16, tag="w")
                            nc.sync.dma_start(wt[:], w_dram[l][k])
                            rhs = ch[cidx][:, fl * NB:(fl + 1) * NB]
                            for m in range(n_m):
                                nc.tensor.matmul(
                                    psums[m][:],
                                    wt[:, m * 128:(m + 1) * 128],
                                    rhs,
                                    start=(k == 0), stop=(k == n_k - 1))

            # ---- drain: scale + bias add, then clip ----
            if l < 2:
                xw_next = [xp.tile([128, F_BLK * NB], FP32, tag="x",
                                   name=f"xn{i}")
                           for i in range(n_m // F_BLK)]
            for m in range(n_m):
                tmp = tmpp.tile([128, NB], FP32, tag="tmp")
                nc.scalar.activation(tmp[:], psums[m][:], AF.Identity,
                                     bias=bias_sb[l][:, m:m + 1],
                                     scale=drain_scale)
                if l < 2:
                    dst = xw_next[m // F_BLK][:, (m % F_BLK) * NB:
                                              (m % F_BLK + 1) * NB]
                    nc.vector.tensor_scalar(dst, tmp[:], 1.0, -1.0,
                                            ALU.min, ALU.max)
                else:
                    o = ostp.tile([128, NB], FP32, tag="ost")
                    nc.vector.tensor_scalar(o[:], tmp[:], 1.0, -1.0,
                                            ALU.min, ALU.max)
                    nc.sync.dma_start(
                        out_dram[m * 128:(m + 1) * 128, nsl], o[:])
            if l < 2:
                xw = xw_next


def _build_program(loop_n=None):
    key = (PREC, loop_n)
    if key in _NC_CACHE:
        return _NC_CACHE[key]

    nc = bacc.Bacc("TRN2", target_bir_lowering=False, debug=False,
                   num_devices=N_CORES)

    xt_dram = nc.dram_tensor("xt", [LAYERS[0], BS], FP32, kind="ExternalInput")
    w_dram, b_dram = [], []
    for l in range(3):
        fin, fout = LAYERS[l], LAYERS[l + 1]
        n_m = fout // 128
        if PREC == "bf16":
            wshape = [(fin // 128) * NCH, 128, n_m * 128]
            wdt = BF16
        else:
            wshape = [(fin // 256) * NCH, 128, 2, n_m * 128]
            wdt = F8
        w_dram.append(nc.dram_tensor(f"w{l}", wshape, wdt, kind="ExternalInput"))
        b_dram.append(nc.dram_tensor(f"b{l}", [128, n_m], FP32,
                                     kind="ExternalInput"))
    out_dram = nc.dram_tensor("out", [LAYERS[3], BS], FP32, kind="ExternalOutput")

    with tile.TileContext(nc) as tc:
        with (
            tc.tile_pool(name="xp", bufs=8) as xp,
            tc.tile_pool(name="chp", bufs=24) as chp,
            tc.tile_pool(name="wp", bufs=8) as wp,
            tc.tile_pool(name="tmpp", bufs=6) as tmpp,
            tc.tile_pool(name="ostp", bufs=4) as ostp,
            tc.tile_pool(name="biasp", bufs=3) as biasp,
            tc.tile_pool(name="psump", bufs=8, space="PSUM") as psump,
        ):
            # const bias tiles for ACT relu shifts (-1, -2, -3)
            cmap = {}
            for v in (-1.0, -2.0, -3.0):
                ct = biasp.tile([128, 1], FP32, name=f"cn{int(-v)}", tag="const")
                nc.vector.memset(ct[:], v)
                cmap[v] = ct

            bias_sb = []
            for l in range(3):
                n_m = LAYERS[l + 1] // 128
                bt = biasp.tile([128, n_m], FP32, tag="bias", name=f"bias{l}")
                nc.sync.dma_start(bt[:], b_dram[l][:])
                bias_sb.append(bt)

            pools = dict(xp=xp, chp=chp, wp=wp, tmpp=tmpp, ostp=ostp,
                         psump=psump)
            tensors = dict(xt=xt_dram, w=w_dram, out=out_dram,
                           bias_sb=bias_sb, cmap=cmap)
            if loop_n:
                with tc.For_i(0, loop_n, 1):
                    _emit_body(nc, pools, tensors)
            else:
                _emit_body(nc, pools, tensors)

    nc.compile()
    _NC_CACHE[key] = nc
    return nc


def _make_in_maps(x, folded):
    in_maps = []
    for core in range(N_CORES):
        shard = x[core * BS:(core + 1) * BS]
        m = {"xt": np.ascontiguousarray(shard.T)}
        for l in range(3):
            m[f"w{l}"] = folded[l][0]
            m[f"b{l}"] = folded[l][1]
        in_maps.append(m)
    return in_maps


# ---------------- entry point ----------------
def kernel(x, base_w0, spline_w0, base_w1, spline_w1, base_w2, spline_w2):
    x = np.asarray(x, dtype=np.float32)
    folded = [
        _fold_weights(np.asarray(base_w0), np.asarray(spline_w0)),
        _fold_weights(np.asarray(base_w1), np.asarray(spline_w1)),
        _fold_weights(np.asarray(base_w2), np.asarray(spline_w2)),
    ]
    nc = _build_program()
    in_maps = _make_in_maps(x, folded)
    res = run_bass_kernel_spmd(nc, in_maps, list(range(N_CORES)))
    out = np.concatenate(
        [np.ascontiguousarray(res.results[i]["out"].T) for i in range(N_CORES)],
        axis=0)
    return out.astype(np.float32)
